# revision 3
# baseline (speedup 1.0000x reference)
"""Trainium2 Bass kernel for quantum-projection multi-head self-attention.

Reference computation (per batch b, head h, with D = 64, H = 16):
    proj = cos(x_heads + theta)                         # [S, D]
    S    = proj @ proj.T / sqrt(D)                      # [S, S]  (symmetric!)
    attn = softmax(S, axis=-1) @ proj                   # [S, D]

Sharding: the 64 (b, h) pairs are data-parallel; 8 pairs per NeuronCore.

Device-side plan per head (S = 2048, D = 64):
  1. DMA x[h] in natural layout as [128, 16*64] (partition = s mod 128).
  2. DVE: w = x/(2pi) + (theta + pi/2)/(2pi); u = w - round(w)  (round via
     +/- 1.5*2^23 trick), so 2*pi*u == x + theta + pi/2 wrapped to [-pi, pi].
  3. ACT: proj = Sin(2*pi*u)  == cos(x + theta).  (Sin spline covers |x|<4.)
  4. PE transposes proj tiles -> projT [64, 2048]; SBUF->SBUF DMA duplicates
     into partitions 64..127 so K=64 matmuls can be packed 2x via row groups.
  5. QK: G[si, :] = projT[:, si].T @ projT  (K=64, fp32), two row-halves run
     concurrently on the 128x128 PE array, PSUM slab [128, 2048].
  6. ACT: E = Exp(G / 8) -> bf16 SBUF slab; free accum_out gives
     Z[si] = sum_t E[si, t] (softmax denominator, fp32).
  7. PV: attn_num[si] = sum_tj E[tj, si-block].T-free (symmetry: the needed
     lhsT tile [t, s] IS E_slab[tj][:, si-block]) @ proj_bf16[tj], PSUM accum.
  8. DVE: out = attn_num * (1/Z) (per-partition scalar), DMA out.

ACT (exp of S^2 elements) is the bottleneck engine; sins are batched in
groups of GROUP heads so the Sin<->Exp activation-table switches amortize.
"""

import math

import numpy as np

import concourse.bass as bass
import concourse.mybir as mybir
import concourse.tile as tile
from concourse import bacc
from concourse.masks import make_identity

AF = mybir.ActivationFunctionType
ALU = mybir.AluOpType

B, S, E = 4, 2048, 1024
H = 16
D = E // H          # 64
N_CORES = 8
HEADS_PER_CORE = (B * H) // N_CORES  # 8

P = 128             # partitions
MAGIC = 1.5 * 2.0**23   # fp32 round-to-nearest trick constant
TWO_PI = 2.0 * math.pi


def build_core_program(s=S, d=D, heads=HEADS_PER_CORE, group=2):
    """Build the single-core Bass program (same NEFF runs SPMD on all cores).

    Returns the compiled-ready Bacc instance. Input DRAM tensors:
      xs : [heads, s, d] fp32   (per-core stack of per-head x slices)
      tb : [P, (s//P)*d] fp32   ((theta + pi/2)/(2pi), tiled along free dim)
    Output:
      out: [heads, s, d] fp32
    """
    n_sblk = s // P                   # 16 query blocks of 128 rows
    nd = n_sblk * d                   # free width of natural-layout tile
    assert s % P == 0 and d == 64

    nc = bacc.Bacc("TRN2", target_bir_lowering=False, debug=False)

    xs = nc.dram_tensor("xs", [heads, s, d], mybir.dt.float32, kind="ExternalInput")
    tb = nc.dram_tensor("tb", [P, nd], mybir.dt.float32, kind="ExternalInput")
    out = nc.dram_tensor("out", [heads, s, d], mybir.dt.float32, kind="ExternalOutput")

    from contextlib import ExitStack

    with tile.TileContext(nc) as tc, ExitStack() as ctx:
        const = ctx.enter_context(tc.tile_pool(name="const", bufs=1))
        sb = ctx.enter_context(tc.tile_pool(name="sb", bufs=2))
        epool = ctx.enter_context(tc.tile_pool(name="epool", bufs=18))
        ps = ctx.enter_context(tc.tile_pool(name="ps", bufs=1, space="PSUM"))

        ident = const.tile([P, P], mybir.dt.float32, tag="ident")
        make_identity(nc, ident)
        tb_sb = const.tile([P, nd], mybir.dt.float32, tag="tb")
        nc.sync.dma_start(tb_sb, tb[:, :])

        n_groups = (heads + group - 1) // group
        for g in range(n_groups):
            hs = list(range(g * group, min((g + 1) * group, heads)))

            projT2s = {}
            pv_rhss = {}
            # ---- sin phase (batched per group: one Sin table residency) ----
            for h in hs:
                x_t = sb.tile([P, nd], mybir.dt.float32, tag="xt", bufs=group + 1)
                nc.sync.dma_start(
                    x_t.rearrange("p (n d) -> p n d", d=d),
                    xs[h].rearrange("(n p) d -> p n d", p=P),
                )
                w = sb.tile([P, nd], mybir.dt.float32, tag="w", bufs=2)
                # w = x * (1/2pi) + tb
                nc.vector.scalar_tensor_tensor(
                    w, x_t, 1.0 / TWO_PI, tb_sb, op0=ALU.mult, op1=ALU.add
                )
                r = sb.tile([P, nd], mybir.dt.float32, tag="r", bufs=2)
                # r = round(w)  via (w + 1.5*2^23) - 1.5*2^23
                nc.vector.tensor_scalar(
                    r, w, MAGIC, MAGIC, op0=ALU.add, op1=ALU.subtract
                )
                u = sb.tile([P, nd], mybir.dt.float32, tag="u", bufs=2)
                nc.vector.tensor_tensor(u, w, r, op=ALU.subtract)
                pn = sb.tile([P, nd], mybir.dt.float32, tag="pn", bufs=group + 1)
                # proj = sin(2pi * u) == cos(x + theta)
                nc.scalar.activation(pn, u, AF.Sin, scale=TWO_PI)

                pv = sb.tile([P, nd], mybir.dt.bfloat16, tag="pv", bufs=group + 1)
                nc.vector.tensor_copy(pv, pn)

                pt = sb.tile([P, s], mybir.dt.float32, tag="pt", bufs=group + 1)
                for n in range(n_sblk):
                    pst = ps.tile([d, P], mybir.dt.float32, tag="T", bufs=2)
                    nc.tensor.transpose(pst, pn[:, n * d:(n + 1) * d], ident)
                    nc.vector.tensor_copy(pt[0:d, n * P:(n + 1) * P], pst)
                # duplicate into partitions 64..127 (SBUF->SBUF DMA; DVE
                # cannot move data across partitions)
                nc.sync.dma_start(pt[d:2 * d, :], pt[0:d, :])
                projT2s[h] = pt
                pv_rhss[h] = pv

            # ---- attention phase ----
            for h in hs:
                pt = projT2s[h]
                pv = pv_rhss[h]
                z = sb.tile([P, n_sblk], mybir.dt.float32, tag="z", bufs=2)
                slabs = []
                for si in range(n_sblk):
                    psS = ps.tile([P, s], mybir.dt.float32, tag="S", bufs=1)
                    # two K=64 row-halves run concurrently on the PE array
                    for nj in range(s // 512):
                        half = nj % 2
                        lo, hi = (0, d) if half == 0 else (d, 2 * d)
                        nc.tensor.matmul(
                            psS[:, nj * 512:(nj + 1) * 512],
                            pt[lo:hi, si * P:(si + 1) * P],
                            pt[lo:hi, nj * 512:(nj + 1) * 512],
                            start=True,
                            stop=True,
                        )
                    e_slab = epool.tile([P, s], mybir.dt.bfloat16, tag="E")
                    nc.scalar.activation(
                        e_slab, psS, AF.Exp,
                        scale=1.0 / math.sqrt(d),
                        accum_out=z[:, si:si + 1],
                    )
                    slabs.append(e_slab)

                rz = sb.tile([P, n_sblk], mybir.dt.float32, tag="rz", bufs=2)
                nc.vector.reciprocal(rz, z)

                for si in range(n_sblk):
                    psO = ps.tile([P, d], mybir.dt.float32, tag="O", bufs=2)
                    for tj in range(n_sblk):
                        # lhsT tile [t, s] == E_slab[tj][:, si-block] (E symmetric)
                        nc.tensor.matmul(
                            psO,
                            slabs[tj][:, si * P:(si + 1) * P],
                            pv[:, tj * d:(tj + 1) * d],
                            start=(tj == 0),
                            stop=(tj == n_sblk - 1),
                        )
                    o_sb = sb.tile([P, d], mybir.dt.float32, tag="os", bufs=4)
                    nc.vector.tensor_scalar_mul(o_sb, psO, rz[:, si:si + 1])
                    nc.sync.dma_start(out[h, si * P:(si + 1) * P, :], o_sb)

    nc.compile()
    return nc


_NC_CACHE = {}


def _get_program(key, **kw):
    if key not in _NC_CACHE:
        _NC_CACHE[key] = build_core_program(**kw)
    return _NC_CACHE[key]


def kernel(x: np.ndarray, mask: np.ndarray, theta: np.ndarray) -> np.ndarray:
    """Full-input entry point: shard across 8 NeuronCores, run, gather."""
    from concourse import bass_utils

    assert x.shape == (B, S, E) and theta.shape == (D,)
    # mask is all-False by construction (fill: zeros); attention is unmasked.

    nc = _get_program("full")

    # [B, S, H, D] -> [B*H, S, D] contiguous per-head slabs
    xh = np.ascontiguousarray(
        x.reshape(B, S, H, D).transpose(0, 2, 1, 3)
    ).reshape(B * H, S, D)

    n_sblk = S // P
    tbv = ((theta + math.pi / 2.0) / TWO_PI).astype(np.float32)  # [D]
    tb = np.broadcast_to(
        np.tile(tbv, n_sblk)[None, :], (P, n_sblk * D)
    ).copy()

    in_maps = [
        {
            "xs": np.ascontiguousarray(
                xh[c * HEADS_PER_CORE:(c + 1) * HEADS_PER_CORE]
            ),
            "tb": tb,
        }
        for c in range(N_CORES)
    ]

    global _last_in_maps
    _last_in_maps = in_maps
    res = bass_utils.run_bass_kernel_spmd(nc, in_maps, core_ids=list(range(N_CORES)))
    outs = [res.results[c]["out"] for c in range(N_CORES)]
    full = np.concatenate(outs, axis=0)  # [B*H, S, D]
    return np.ascontiguousarray(
        full.reshape(B, H, S, D).transpose(0, 2, 1, 3)
    ).reshape(B, S, E)


# revision 8
# speedup vs baseline: 1.6075x; 1.6075x over previous
"""Trainium2 Bass kernel for quantum-projection multi-head self-attention.

Reference computation (per batch b, head h, with D = 64, H = 16):
    proj = cos(x_heads + theta)                         # [S, D]
    S    = proj @ proj.T / sqrt(D)                      # [S, S]  (symmetric!)
    attn = softmax(S, axis=-1) @ proj                   # [S, D]

Sharding: the 64 (b, h) pairs are data-parallel; 8 pairs per NeuronCore.

Device-side plan per head (S = 2048, D = 64):
  1. DMA x[h] in natural layout as [128, 16*64] (partition = s mod 128).
  2. DVE: w = x/(2pi) + (theta + pi/2)/(2pi); u = w - round(w)  (round via
     +/- 1.5*2^23 trick), so 2*pi*u == x + theta + pi/2 wrapped to [-pi, pi].
  3. ACT: proj = Sin(2*pi*u)  == cos(x + theta).  (Sin spline covers |x|<4.)
  4. PE transposes proj tiles -> projT [64, 2048]; SBUF->SBUF DMA duplicates
     into partitions 64..127 so K=64 matmuls can be packed 2x via row groups.
  5. QK: G[si, :] = projT[:, si].T @ projT  (K=64, fp32), two row-halves run
     concurrently on the 128x128 PE array, PSUM slab [128, 2048].
  6. ACT: E = Exp(G / 8) -> bf16 SBUF slab; free accum_out gives
     Z[si] = sum_t E[si, t] (softmax denominator, fp32).
  7. PV: attn_num[si] = sum_tj E[tj, si-block].T-free (symmetry: the needed
     lhsT tile [t, s] IS E_slab[tj][:, si-block]) @ proj_bf16[tj], PSUM accum.
  8. DVE: out = attn_num * (1/Z) (per-partition scalar), DMA out.

ACT (exp of S^2 elements) is the bottleneck engine; sins are batched in
groups of GROUP heads so the Sin<->Exp activation-table switches amortize.
"""

import math

import numpy as np

import concourse.bass as bass
import concourse.mybir as mybir
import concourse.tile as tile
from concourse import bacc
from concourse.masks import make_identity

AF = mybir.ActivationFunctionType
ALU = mybir.AluOpType

B, S, E = 4, 2048, 1024
H = 16
D = E // H          # 64
N_CORES = 8
HEADS_PER_CORE = (B * H) // N_CORES  # 8

P = 128             # partitions
MAGIC = 1.5 * 2.0**23   # fp32 round-to-nearest trick constant
TWO_PI = 2.0 * math.pi


def build_core_program(s=S, d=D, heads=HEADS_PER_CORE, group=2):
    """Build the single-core Bass program (same NEFF runs SPMD on all cores).

    Returns the compiled-ready Bacc instance. Input DRAM tensors:
      xs : [heads, s, d] fp32   (per-core stack of per-head x slices)
      tb : [P, (s//P)*d] fp32   ((theta + pi/2)/(2pi), tiled along free dim)
    Output:
      out: [heads, s, d] fp32
    """
    n_sblk = s // P                   # 16 query blocks of 128 rows
    nd = n_sblk * d                   # free width of natural-layout tile
    d1 = d + 1                        # PV rhs width incl. ones column (Z)
    assert s % P == 0 and d == 64

    nc = bacc.Bacc("TRN2", target_bir_lowering=False, debug=False)

    xs = nc.dram_tensor("xs", [heads, s, d], mybir.dt.float32, kind="ExternalInput")
    tb = nc.dram_tensor("tb", [P, nd], mybir.dt.float32, kind="ExternalInput")
    out = nc.dram_tensor("out", [heads, s, d], mybir.dt.float32, kind="ExternalOutput")

    from contextlib import ExitStack

    with tile.TileContext(nc) as tc, ExitStack() as ctx:
        const = ctx.enter_context(tc.tile_pool(name="const", bufs=1))
        sb = ctx.enter_context(tc.tile_pool(name="sb", bufs=2))
        epool = ctx.enter_context(tc.tile_pool(name="epool", bufs=18))
        ps = ctx.enter_context(tc.tile_pool(name="ps", bufs=1, space="PSUM"))

        ident = const.tile([P, P], mybir.dt.bfloat16, tag="ident")
        make_identity(nc, ident)
        tb_sb = const.tile([P, nd], mybir.dt.float32, tag="tb")
        nc.sync.dma_start(tb_sb, tb[:, :])

        n_groups = (heads + group - 1) // group
        for g in range(n_groups):
            hs = list(range(g * group, min((g + 1) * group, heads)))

            projT2s = {}
            pv_rhss = {}
            # ---- sin phase (batched per group: one Sin table residency) ----
            for h in hs:
                x_t = sb.tile([P, nd], mybir.dt.float32, tag="xt", bufs=group + 1)
                nc.sync.dma_start(
                    x_t.rearrange("p (n d) -> p n d", d=d),
                    xs[h].rearrange("(n p) d -> p n d", p=P),
                )
                w = sb.tile([P, nd], mybir.dt.float32, tag="w", bufs=2)
                # w = x * (1/2pi) + tb
                nc.vector.scalar_tensor_tensor(
                    w, x_t, 1.0 / TWO_PI, tb_sb, op0=ALU.mult, op1=ALU.add
                )
                r = sb.tile([P, nd], mybir.dt.float32, tag="r", bufs=2)
                # r = round(w)  via (w + 1.5*2^23) - 1.5*2^23
                nc.vector.tensor_scalar(
                    r, w, MAGIC, MAGIC, op0=ALU.add, op1=ALU.subtract
                )
                u = sb.tile([P, nd], mybir.dt.float32, tag="u", bufs=2)
                nc.vector.tensor_tensor(u, w, r, op=ALU.subtract)
                # pvx holds proj in bf16 with a 1.0 column appended per
                # d-group: [128, 16*(64+1)]; the ones column makes the PV
                # matmul also produce Z = sum_t E[s, t] in its 65th column.
                pvx = sb.tile([P, n_sblk * d1], mybir.dt.bfloat16,
                              tag="pvx", bufs=group + 1)
                ones_view = pvx.rearrange("p (n e) -> p n e", e=d1)[:, :, d:d1]
                nc.vector.memset(ones_view, 1.0)
                pv = pvx.rearrange("p (n e) -> p n e", e=d1)[:, :, 0:d]
                # proj = sin(2pi * u) == cos(x + theta), written bf16 directly
                # into the strided [128, (16, 64)] view
                nc.scalar.activation(pv, u.rearrange("p (n e) -> p n e", e=d),
                                     AF.Sin, scale=TWO_PI)

                pt = sb.tile([P, s], mybir.dt.bfloat16, tag="pt", bufs=group + 1)
                for n in range(n_sblk):
                    pst = ps.tile([d, P], mybir.dt.bfloat16, tag="T", bufs=2)
                    nc.tensor.transpose(pst, pv[:, n, :], ident)
                    nc.vector.tensor_copy(pt[0:d, n * P:(n + 1) * P], pst)
                # duplicate into partitions 64..127 (SBUF->SBUF DMA; DVE
                # cannot move data across partitions)
                nc.sync.dma_start(pt[d:2 * d, :], pt[0:d, :])
                projT2s[h] = pt
                pv_rhss[h] = pvx

            # ---- attention phase ----
            for h in hs:
                pt = projT2s[h]
                pvx = pv_rhss[h]
                slabs = []
                for si in range(n_sblk):
                    psS = ps.tile([P, s], mybir.dt.float32, tag="S", bufs=1)
                    # two K=64 row-halves run concurrently on the PE array
                    for nj in range(s // 512):
                        half = nj % 2
                        lo, hi = (0, d) if half == 0 else (d, 2 * d)
                        nc.tensor.matmul(
                            psS[:, nj * 512:(nj + 1) * 512],
                            pt[lo:hi, si * P:(si + 1) * P],
                            pt[lo:hi, nj * 512:(nj + 1) * 512],
                            start=True,
                            stop=True,
                        )
                    e_slab = epool.tile([P, s], mybir.dt.bfloat16, tag="E")
                    nc.scalar.activation(
                        e_slab, psS, AF.Exp, scale=1.0 / math.sqrt(d)
                    )
                    slabs.append(e_slab)

                for si in range(n_sblk):
                    psO = ps.tile([P, d1], mybir.dt.float32, tag="O", bufs=2)
                    for tj in range(n_sblk):
                        # lhsT tile [t, s] == E_slab[tj][:, si-block] (E symmetric);
                        # rhs includes the ones column -> psO[:, 64] = Z
                        nc.tensor.matmul(
                            psO,
                            slabs[tj][:, si * P:(si + 1) * P],
                            pvx[:, tj * d1:(tj + 1) * d1],
                            start=(tj == 0),
                            stop=(tj == n_sblk - 1),
                        )
                    rz = sb.tile([P, 1], mybir.dt.float32, tag="rz", bufs=4)
                    nc.vector.reciprocal(rz, psO[:, d:d1])
                    o_sb = sb.tile([P, d], mybir.dt.float32, tag="os", bufs=4)
                    nc.vector.tensor_scalar_mul(o_sb, psO[:, 0:d], rz)
                    nc.sync.dma_start(out[h, si * P:(si + 1) * P, :], o_sb)

    nc.compile()
    return nc


_NC_CACHE = {}


def _get_program(key, **kw):
    if key not in _NC_CACHE:
        _NC_CACHE[key] = build_core_program(**kw)
    return _NC_CACHE[key]


def kernel(x: np.ndarray, mask: np.ndarray, theta: np.ndarray) -> np.ndarray:
    """Full-input entry point: shard across 8 NeuronCores, run, gather."""
    from concourse import bass_utils

    assert x.shape == (B, S, E) and theta.shape == (D,)
    # mask is all-False by construction (fill: zeros); attention is unmasked.

    nc = _get_program("full")

    # [B, S, H, D] -> [B*H, S, D] contiguous per-head slabs
    xh = np.ascontiguousarray(
        x.reshape(B, S, H, D).transpose(0, 2, 1, 3)
    ).reshape(B * H, S, D)

    n_sblk = S // P
    tbv = ((theta + math.pi / 2.0) / TWO_PI).astype(np.float32)  # [D]
    tb = np.broadcast_to(
        np.tile(tbv, n_sblk)[None, :], (P, n_sblk * D)
    ).copy()

    in_maps = [
        {
            "xs": np.ascontiguousarray(
                xh[c * HEADS_PER_CORE:(c + 1) * HEADS_PER_CORE]
            ),
            "tb": tb,
        }
        for c in range(N_CORES)
    ]

    global _last_in_maps
    _last_in_maps = in_maps
    res = bass_utils.run_bass_kernel_spmd(nc, in_maps, core_ids=list(range(N_CORES)))
    outs = [res.results[c]["out"] for c in range(N_CORES)]
    full = np.concatenate(outs, axis=0)  # [B*H, S, D]
    return np.ascontiguousarray(
        full.reshape(B, H, S, D).transpose(0, 2, 1, 3)
    ).reshape(B, S, E)
